# revision 37
# baseline (speedup 1.0000x reference)
"""Multi-head causal attention (B=2, S=2048, HID=2048, H=16, D=128) on 8 TRN2
NeuronCores.

Sharding: core c handles batch b=c//4 and heads [4*(c%4) .. 4*(c%4)+3].
Each core computes qkv-projection + RoPE + causal attention + its partial
out-projection; the host sums the 4 partial outputs per batch (tensor-parallel
reduce) and stacks the 2 batches.

All matmul operands are bf16 (PSUM accumulation stays f32); the PE streams
bf16 at full rate and DMA bytes halve. Host pre-arranges every tensor into
its exact SBUF layout so each load is one flat DMA.

Phases (single TileContext, per-engine in-order queues):
  1. merged QKV projection + RoPE: one pass over x per 512-token block;
     V uses x-stationary matmuls ([tok, head*d] layout), Q/K use
     w-stationary ([d, tok]) with RoPE applied during PSUM evacuation.
  2. attention per (head, q-block): S^T pairs -> exp (causal mask folded
     into the S accumulation as an identity-lhsT matmul adding a -1e9
     staircase) -> O/V accumulation; row sums via ones-matmul on
     vector-pre-summed A pairs; software-pipelined so the tensor engine
     never waits on the scalar exp.
  3. out projection: accumulate 4 heads per 128-token chunk, DMA out.
"""
import sys

sys.path.insert(0, '/opt/trn_rl_repo')

import numpy as np

B, S, HID = 2, 2048, 2048
H, D = 16, 128
NH = H // 4          # heads per core = 4
HC = HID // 128      # hid chunks = 16
TB = 512             # token block for projection
NTB = S // TB        # 4
QB = 512             # q block in attention
NQB = S // QB        # 4
SCALE = 1.0 / float(np.sqrt(D))
BASE = 10000.0
NEG = -1.0e9
N_CORES = 8

_cache = {}


def _build():
    import concourse.bass as bass  # noqa: F401
    import concourse.tile as tile
    from concourse import bacc, mybir

    f32 = mybir.dt.float32
    bf16 = mybir.dt.bfloat16
    EXP = mybir.ActivationFunctionType.Exp
    MULT = mybir.AluOpType.mult
    ADD = mybir.AluOpType.add

    nc = bacc.Bacc("TRN2", target_bir_lowering=False, debug=False,
                   num_devices=N_CORES)

    xA = nc.dram_tensor("xA", [128, HC * S], bf16, kind="ExternalInput").ap()
    # xA layout is per-jb-block contiguous: [p, jb*HC*TB + c*TB + t]
    wqkA = nc.dram_tensor("wqkA", [128, HC * 8 * 128], bf16,
                          kind="ExternalInput").ap()
    wvA = nc.dram_tensor("wvA", [128, HC * NH * 128], bf16,
                         kind="ExternalInput").ap()
    woA = nc.dram_tensor("woA", [128, NH * HID], bf16,
                         kind="ExternalInput").ap()
    cosT = nc.dram_tensor("cosT", [128, S], f32, kind="ExternalInput").ap()
    sinS = nc.dram_tensor("sinS", [128, S], f32, kind="ExternalInput").ap()
    maskM = nc.dram_tensor("maskM", [128, 4 * QB], bf16,
                           kind="ExternalInput").ap()
    identM = nc.dram_tensor("identM", [128, 128], bf16,
                            kind="ExternalInput").ap()
    ones_sq = nc.dram_tensor("ones_sq", [128, 128], bf16,
                             kind="ExternalInput").ap()
    y = nc.dram_tensor("y", [S, HID], bf16, kind="ExternalOutput").ap()

    with tile.TileContext(nc) as tc:
        with tc.tile_pool(name="persist", bufs=1) as pp:
            # ---- resident tensors ----
            xTb_n = HC * TB
            wqk_sb = pp.tile([128, HC * 1024], bf16, tag="wqk_sb")
            wv_sb = pp.tile([128, HC * 512], bf16, tag="wv_sb")
            wo_sb = pp.tile([128, NH * HID], bf16, tag="wo_sb")
            tcos = pp.tile([128, S], f32, tag="tcos")
            tsin = pp.tile([128, S], f32, tag="tsin")
            tmask = pp.tile([128, 4 * QB], bf16, tag="tmask")
            tid = pp.tile([128, 128], bf16, tag="tid")
            t1s = pp.tile([128, 128], bf16, tag="t1s")
            qkT = [pp.tile([128, S], bf16, tag=f"qkT{i}", name=f"qkT{i}")
                   for i in range(8)]
            v_all = pp.tile([128, HC * NH * 128], bf16, tag="v_all")
            outT = [pp.tile([128, S], bf16, tag=f"outT{h}", name=f"outT{h}")
                    for h in range(NH)]

            # ---- phase 1: merged V + QK projection with RoPE ----
            with tc.tile_pool(name="p1x", bufs=2) as p1x, \
                 tc.tile_pool(name="rope", bufs=3) as rp, \
                 tc.tile_pool(name="psP", bufs=4, space="PSUM") as psP:
                # issue order = first-use order; quartered so the first
                # accumulation groups start on the first arrivals
                xTb0 = p1x.tile([128, xTb_n], bf16, tag="xTb")
                for sp in range(8):
                    xs = slice(sp * 2 * TB, (sp + 1) * 2 * TB)
                    ws = slice(sp * 2 * 512, (sp + 1) * 2 * 512)
                    nc.sync.dma_start(xTb0[:, xs], xA[:, xs])
                    nc.sync.dma_start(wv_sb[:, ws], wvA[:, ws])
                for sp in range(4):
                    qs = slice(sp * 4 * 1024, (sp + 1) * 4 * 1024)
                    nc.sync.dma_start(wqk_sb[:, qs], wqkA[:, qs])
                nc.sync.dma_start(tcos[:], cosT[:])
                nc.sync.dma_start(tsin[:], sinS[:])

                xTb_cur = xTb0
                for jb in range(NTB):
                    # prefetch next x block
                    if jb + 1 < NTB:
                        xTb_nxt = p1x.tile([128, xTb_n], bf16, tag="xTb")
                        nc.sync.dma_start(
                            xTb_nxt[:],
                            xA[:, (jb + 1) * xTb_n:(jb + 2) * xTb_n])
                    if jb == 1:
                        # later-phase loads, queued after the jb prefetches
                        nc.sync.dma_start(tmask[:], maskM[:])
                        nc.sync.dma_start(tid[:], identM[:])
                        nc.sync.dma_start(t1s[:], ones_sq[:])
                        nc.sync.dma_start(wo_sb[:], woA[:])
                    xTb = xTb_cur
                    sl = slice(jb * TB, (jb + 1) * TB)

                    def v_mm(Pv, t2, c):
                        nc.tensor.matmul(
                            Pv[:],
                            xTb[:, c * TB + t2 * 128:c * TB + (t2 + 1) * 128],
                            wv_sb[:, c * 512:(c + 1) * 512],
                            start=(c == 0), stop=(c == HC - 1))

                    def v_evac(Pv, t2):
                        cg = jb * (TB // 128) + t2
                        nc.scalar.copy(v_all[:, cg * 512:(cg + 1) * 512],
                                       Pv[:])

                    def qk_mm(P, cc, c):
                        nc.tensor.matmul(
                            P[:],
                            wqk_sb[:, c * 1024 + cc * 128:
                                   c * 1024 + (cc + 1) * 128],
                            xTb[:, c * TB:(c + 1) * TB],
                            start=(c == 0), stop=(c == HC - 1))

                    def qk_rope(P, cc):
                        u = rp.tile([128, TB], f32, tag="u")
                        nc.scalar.copy(u[:], P[:])
                        rot = rp.tile([128, TB], f32, tag="rot")
                        nc.sync.dma_start(rot[0:64, :], u[64:128, :])
                        nc.sync.dma_start(rot[64:128, :], u[0:64, :])
                        m = rp.tile([128, TB], f32, tag="m")
                        nc.vector.tensor_tensor(
                            out=m[:], in0=rot[:], in1=tsin[:, sl], op=MULT)
                        t = rp.tile([128, TB], f32, tag="t")
                        nc.vector.tensor_tensor(
                            out=t[:], in0=u[:], in1=tcos[:, sl], op=MULT)
                        nc.vector.tensor_tensor(
                            out=qkT[cc][:, sl], in0=t[:], in1=m[:], op=ADD)

                    if jb == 0:
                        # consume hid-chunk quarters in DMA-arrival order
                        Pvs = [psP.tile([128, 512], f32, tag="P",
                                        name=f"Pv{t2}")
                               for t2 in range(4)]
                        for cq in range(4):
                            for t2 in range(4):
                                for c in range(4 * cq, 4 * cq + 4):
                                    v_mm(Pvs[t2], t2, c)
                                if cq == 3:
                                    v_evac(Pvs[t2], t2)
                        for half in (0, 1):
                            Ps = [psP.tile([128, TB], f32, tag="P",
                                           name=f"Pqk{half}{i}")
                                  for i in range(4)]
                            for cq in range(4):
                                for i in range(4):
                                    cc = 4 * half + i
                                    for c in range(4 * cq, 4 * cq + 4):
                                        qk_mm(Ps[i], cc, c)
                                    if cq == 3:
                                        qk_rope(Ps[i], cc)
                    else:
                        for t2 in range(TB // 128):
                            Pv = psP.tile([128, 512], f32, tag="P")
                            for c in range(HC):
                                v_mm(Pv, t2, c)
                            v_evac(Pv, t2)
                        for cc in range(8):
                            P = psP.tile([128, TB], f32, tag="P")
                            for c in range(HC):
                                qk_mm(P, cc, c)
                            qk_rope(P, cc)
                    xTb_cur = xTb_nxt if jb + 1 < NTB else None

            # ---- phases 2+3 interleaved: attention (two head-units in
            # flight), then the finished q-block's out-projection ----
            with tc.tile_pool(name="p2a", bufs=6) as p2a, \
                 tc.tile_pool(name="p2s", bufs=4) as p2s, \
                 tc.tile_pool(name="p2r", bufs=2) as p2r, \
                 tc.tile_pool(name="psS", bufs=2, space="PSUM") as psS, \
                 tc.tile_pool(name="psO", bufs=2, space="PSUM") as psO, \
                 tc.tile_pool(name="psR", bufs=2, space="PSUM") as psR:

                def emit_S(h, jb4, kc, tag):
                    """One S^T chunk matmul."""
                    qT_h, kT_h = qkT[h], qkT[NH + h]
                    qsl = slice(jb4 * QB, (jb4 + 1) * QB)
                    Sc = psS.tile([128, QB], f32, tag=tag)
                    nc.tensor.matmul(
                        Sc[:], kT_h[:, kc * 128:(kc + 1) * 128], qT_h[:, qsl],
                        start=True, stop=True)
                    return Sc

                def emit_exp_O(h, kc, Sc, O, nkc, jb4):
                    md = kc - 4 * jb4
                    A = p2a.tile([128, QB], bf16, tag="A")
                    nc.scalar.activation(A[:], Sc[:], EXP, scale=SCALE)
                    if md >= 0:  # diagonal chunk: 0/1 mask post-exp
                        Am = p2a.tile([128, QB], bf16, tag="A")
                        nc.vector.tensor_tensor(
                            out=Am[:], in0=A[:],
                            in1=tmask[:, md * QB:(md + 1) * QB], op=MULT)
                        A = Am
                    nc.tensor.matmul(
                        O[:],
                        v_all[:, kc * 512 + h * 128:kc * 512 + (h + 1) * 128],
                        A[:],
                        start=(kc == 0), stop=(kc == nkc - 1))
                    return A

                def emit_R(R, Asum, q, nquad):
                    nc.tensor.matmul(R[:], t1s[:], Asum[:],
                                     start=(q == 0), stop=(q == nquad - 1))

                class Unit:
                    def __init__(self, h, jb4, tag):
                        self.h, self.jb4, self.tag = h, jb4, tag
                        self.nkc = 4 * (jb4 + 1)
                        self.O = psO.tile([128, QB], f32, tag="O")
                        self.R = psR.tile([128, QB], f32, tag="R")
                        self.Sc = [emit_S(h, jb4, 0, tag),
                                   emit_S(h, jb4, 1, tag)]
                        self.A0 = None      # first A of current pair
                        self.Asum_pend = None

                    def step(self, kc):
                        """Emit delayed quad-R, exp+O(kc), pair/quad sums on
                        vector, then lookahead S(kc+2) (after exp so the freed
                        psS bank's reuse is clean)."""
                        nquad = self.nkc // 4
                        if kc % 4 == 1 and self.Asum_pend is not None:
                            emit_R(self.R, self.Asum_pend, kc // 4 - 1, nquad)
                            self.Asum_pend = None
                        A = emit_exp_O(self.h, kc, self.Sc.pop(0), self.O,
                                       self.nkc, self.jb4)
                        if kc % 2 == 0:
                            self.A0 = A
                        else:
                            Asum = p2s.tile([128, QB], bf16, tag="Asum")
                            nc.vector.tensor_tensor(
                                out=Asum[:], in0=self.A0[:], in1=A[:], op=ADD)
                            if kc % 4 == 1:
                                self.Ap = Asum      # first pair of the quad
                            else:
                                Asum2 = p2s.tile([128, QB], bf16, tag="Asum")
                                nc.vector.tensor_tensor(
                                    out=Asum2[:], in0=self.Ap[:], in1=Asum[:],
                                    op=ADD)
                                self.Asum_pend = Asum2
                        if kc + 2 < self.nkc:
                            self.Sc.append(
                                emit_S(self.h, self.jb4, kc + 2, self.tag))

                    def finish(self):
                        nquad = self.nkc // 4
                        emit_R(self.R, self.Asum_pend, nquad - 1, nquad)
                        rec = p2r.tile([128, QB], f32, tag="rec")
                        nc.vector.reciprocal_approx_fast(out=rec[:],
                                                         in_=self.R[:])
                        qsl = slice(self.jb4 * QB, (self.jb4 + 1) * QB)
                        nc.vector.tensor_tensor(
                            out=outT[self.h][:, qsl], in0=self.O[:],
                            in1=rec[:], op=MULT)

                for jb4 in range(NQB):
                    nkc = 4 * (jb4 + 1)
                    for h0 in (0, 2):
                        ua = Unit(h0, jb4, "Sa")
                        ub = Unit(h0 + 1, jb4, "Sb")
                        for kc in range(nkc):
                            ua.step(kc)
                            ub.step(kc)
                        ua.finish()
                        ub.finish()

            # ---- phase 3: out projection (partial) ----
            with tc.tile_pool(name="p3", bufs=4) as p3, \
                 tc.tile_pool(name="ps3", bufs=4, space="PSUM") as ps3:
                for tch in range(S // 128):
                    for cb in range(HID // 512):
                        P3 = ps3.tile([128, 512], f32, tag="P3")
                        for h in range(NH):
                            nc.tensor.matmul(
                                P3[:],
                                outT[h][:, tch * 128:(tch + 1) * 128],
                                wo_sb[:, h * HID + cb * 512:
                                      h * HID + (cb + 1) * 512],
                                start=(h == 0), stop=(h == NH - 1))
                        ys = p3.tile([128, 512], bf16, tag="ys")
                        if (tch * 4 + cb) % 2 == 0:
                            nc.vector.tensor_copy(ys[:], P3[:])
                        else:
                            nc.scalar.copy(ys[:], P3[:])
                        nc.sync.dma_start(
                            y[tch * 128:(tch + 1) * 128,
                              cb * 512:(cb + 1) * 512], ys[:])

    nc.compile()
    return nc


def _host_inputs(x, w_qkv, w_out):
    """Build the 8 per-core input maps, pre-arranged into SBUF layouts."""
    import ml_dtypes
    bf16 = ml_dtypes.bfloat16

    # RoPE tables, [d, t] with the rotate-half sign folded into sin.
    inv_freq = 1.0 / (BASE ** (np.arange(0, D, 2, dtype=np.float64) / D))
    pos = np.arange(S, dtype=np.float64)
    freqs = np.outer(inv_freq, pos)           # [64, S]
    cos_h = np.cos(freqs).astype(np.float32)
    sin_h = np.sin(freqs).astype(np.float32)
    cosT = np.concatenate([cos_h, cos_h], 0)  # [128, S]
    sinS = np.concatenate([-sin_h, sin_h], 0)

    # Additive causal masks for the 4 diagonal chunk offsets ([k, q-free]).
    kp = np.arange(128)[:, None]
    qf = np.arange(QB)[None, :]
    maskM = np.concatenate(
        [(qf >= 128 * mm + kp).astype(np.float32) for mm in range(4)],
        axis=1).astype(bf16)

    w3 = np.asarray(w_qkv, np.float32).reshape(HID, 3, H, D)
    wo_full = np.asarray(w_out, np.float32).reshape(H, D, HID)
    x = np.asarray(x, np.float32)

    shared = {
        "cosT": cosT, "sinS": sinS, "maskM": maskM,
        "identM": np.eye(128, dtype=bf16),
        "ones_sq": np.ones((128, 128), bf16),
    }
    in_maps = []
    for c in range(N_CORES):
        b, hg = c // 4, c % 4
        heads = slice(4 * hg, 4 * hg + 4)
        # xA[p, jb*HC*TB + c*TB + t] = x[b, jb*TB+t, c*128+p]
        xA = np.ascontiguousarray(
            x[b].reshape(NTB, TB, HC, 128).transpose(3, 0, 2, 1)
            .reshape(128, HC * S)).astype(bf16)
        # wqkA[p, c*1024 + cc*128 + j]: cc<4 q heads, cc>=4 k heads
        wqk = w3[:, 0:2, heads, :].reshape(HC, 128, 2 * NH * 128)
        wqkA = np.ascontiguousarray(
            wqk.transpose(1, 0, 2).reshape(128, HC * 1024)).astype(bf16)
        # wvA[p, c*512 + hl*128 + j]
        wv = w3[:, 2, heads, :].reshape(HC, 128, NH * 128)
        wvA = np.ascontiguousarray(
            wv.transpose(1, 0, 2).reshape(128, HC * 512)).astype(bf16)
        # woA[p, hl*HID + n] = wo_full[4hg+hl, p, n]
        woA = np.ascontiguousarray(
            wo_full[heads].transpose(1, 0, 2).reshape(128, NH * HID)
        ).astype(bf16)
        in_maps.append({
            "xA": xA, "wqkA": wqkA, "wvA": wvA, "woA": woA, **shared,
        })
    return in_maps


def kernel(x, w_qkv, w_out):
    from concourse.bass_utils import run_bass_kernel_spmd

    if "nc" not in _cache:
        _cache["nc"] = _build()
    nc = _cache["nc"]
    in_maps = _host_inputs(x, w_qkv, w_out)
    res = run_bass_kernel_spmd(nc, in_maps, core_ids=list(range(N_CORES)))
    out = np.zeros((B, S, HID), np.float32)
    for c in range(N_CORES):
        out[c // 4] += res.results[c]["y"].astype(np.float32)
    return out


# revision 38
# speedup vs baseline: 1.0344x; 1.0344x over previous
"""Multi-head causal attention (B=2, S=2048, HID=2048, H=16, D=128) on 8 TRN2
NeuronCores.

Sharding: core c handles batch b=c//4 and heads [4*(c%4) .. 4*(c%4)+3].
Each core computes qkv-projection + RoPE + causal attention + its partial
out-projection; the host sums the 4 partial outputs per batch (tensor-parallel
reduce) and stacks the 2 batches.

All matmul operands are bf16 (PSUM accumulation stays f32); the PE streams
bf16 at full rate and DMA bytes halve. Host pre-arranges every tensor into
its exact SBUF layout so each load is one flat DMA.

Phases (single TileContext, per-engine in-order queues):
  1. merged QKV projection + RoPE: one pass over x per 512-token block;
     V uses x-stationary matmuls ([tok, head*d] layout), Q/K use
     w-stationary ([d, tok]) with RoPE applied during PSUM evacuation.
  2. attention per (head, q-block): S^T pairs -> exp (causal mask folded
     into the S accumulation as an identity-lhsT matmul adding a -1e9
     staircase) -> O/V accumulation; row sums via ones-matmul on
     vector-pre-summed A pairs; software-pipelined so the tensor engine
     never waits on the scalar exp.
  3. out projection: accumulate 4 heads per 128-token chunk, DMA out.
"""
import sys

sys.path.insert(0, '/opt/trn_rl_repo')

import numpy as np

B, S, HID = 2, 2048, 2048
H, D = 16, 128
NH = H // 4          # heads per core = 4
HC = HID // 128      # hid chunks = 16
TB = 512             # token block for projection
NTB = S // TB        # 4
QB = 512             # q block in attention
NQB = S // QB        # 4
SCALE = 1.0 / float(np.sqrt(D))
BASE = 10000.0
NEG = -1.0e9
N_CORES = 8

_cache = {}


def _build():
    import concourse.bass as bass  # noqa: F401
    import concourse.tile as tile
    from concourse import bacc, mybir

    f32 = mybir.dt.float32
    bf16 = mybir.dt.bfloat16
    EXP = mybir.ActivationFunctionType.Exp
    MULT = mybir.AluOpType.mult
    ADD = mybir.AluOpType.add

    nc = bacc.Bacc("TRN2", target_bir_lowering=False, debug=False,
                   num_devices=N_CORES)

    xA = nc.dram_tensor("xA", [128, HC * S], bf16, kind="ExternalInput").ap()
    # xA layout is per-jb-block contiguous: [p, jb*HC*TB + c*TB + t]
    wqkA = nc.dram_tensor("wqkA", [128, HC * 8 * 128], bf16,
                          kind="ExternalInput").ap()
    wvA = nc.dram_tensor("wvA", [128, HC * NH * 128], bf16,
                         kind="ExternalInput").ap()
    woA = nc.dram_tensor("woA", [128, NH * HID], bf16,
                         kind="ExternalInput").ap()
    cosT = nc.dram_tensor("cosT", [128, S], f32, kind="ExternalInput").ap()
    sinS = nc.dram_tensor("sinS", [128, S], f32, kind="ExternalInput").ap()
    maskM = nc.dram_tensor("maskM", [128, 4 * QB], bf16,
                           kind="ExternalInput").ap()
    identM = nc.dram_tensor("identM", [128, 128], bf16,
                            kind="ExternalInput").ap()
    ones_sq = nc.dram_tensor("ones_sq", [128, 128], bf16,
                             kind="ExternalInput").ap()
    y = nc.dram_tensor("y", [S, HID], bf16, kind="ExternalOutput").ap()

    with tile.TileContext(nc) as tc:
        with tc.tile_pool(name="persist", bufs=1) as pp:
            # ---- resident tensors ----
            xTb_n = HC * TB
            wqk_sb = pp.tile([128, HC * 1024], bf16, tag="wqk_sb")
            wv_sb = pp.tile([128, HC * 512], bf16, tag="wv_sb")
            wo_sb = pp.tile([128, NH * HID], bf16, tag="wo_sb")
            tcos = pp.tile([128, S], f32, tag="tcos")
            tsin = pp.tile([128, S], f32, tag="tsin")
            tmask = pp.tile([128, 4 * QB], bf16, tag="tmask")
            tid = pp.tile([128, 128], bf16, tag="tid")
            t1s = pp.tile([128, 128], bf16, tag="t1s")
            qkT = [pp.tile([128, S], bf16, tag=f"qkT{i}", name=f"qkT{i}")
                   for i in range(8)]
            v_all = pp.tile([128, HC * NH * 128], bf16, tag="v_all")
            outT = [pp.tile([128, S], bf16, tag=f"outT{h}", name=f"outT{h}")
                    for h in range(NH)]

            # ---- phase 1: merged V + QK projection with RoPE ----
            with tc.tile_pool(name="p1x", bufs=2) as p1x, \
                 tc.tile_pool(name="rope", bufs=3) as rp, \
                 tc.tile_pool(name="psP", bufs=4, space="PSUM") as psP:
                # issue order = first-use order; quartered so the first
                # accumulation groups start on the first arrivals
                xTb0 = p1x.tile([128, xTb_n], bf16, tag="xTb")
                for sp in range(8):
                    xs = slice(sp * 2 * TB, (sp + 1) * 2 * TB)
                    ws = slice(sp * 2 * 512, (sp + 1) * 2 * 512)
                    nc.sync.dma_start(xTb0[:, xs], xA[:, xs])
                    nc.sync.dma_start(wv_sb[:, ws], wvA[:, ws])
                for sp in range(4):
                    qs = slice(sp * 4 * 1024, (sp + 1) * 4 * 1024)
                    nc.sync.dma_start(wqk_sb[:, qs], wqkA[:, qs])
                nc.sync.dma_start(tcos[:], cosT[:])
                nc.sync.dma_start(tsin[:], sinS[:])

                xTb_cur = xTb0
                for jb in range(NTB):
                    # prefetch next x block
                    if jb + 1 < NTB:
                        xTb_nxt = p1x.tile([128, xTb_n], bf16, tag="xTb")
                        nc.sync.dma_start(
                            xTb_nxt[:],
                            xA[:, (jb + 1) * xTb_n:(jb + 2) * xTb_n])
                    if jb == 1:
                        # later-phase loads, queued after the jb prefetches
                        nc.sync.dma_start(tmask[:], maskM[:])
                        nc.sync.dma_start(tid[:], identM[:])
                        nc.sync.dma_start(t1s[:], ones_sq[:])
                        nc.sync.dma_start(wo_sb[:], woA[:])
                    xTb = xTb_cur
                    sl = slice(jb * TB, (jb + 1) * TB)

                    def v_mm(Pv, t2, c):
                        nc.tensor.matmul(
                            Pv[:],
                            xTb[:, c * TB + t2 * 128:c * TB + (t2 + 1) * 128],
                            wv_sb[:, c * 512:(c + 1) * 512],
                            start=(c == 0), stop=(c == HC - 1))

                    def v_evac(Pv, t2):
                        cg = jb * (TB // 128) + t2
                        nc.scalar.copy(v_all[:, cg * 512:(cg + 1) * 512],
                                       Pv[:])

                    def qk_mm(P, cc, c):
                        nc.tensor.matmul(
                            P[:],
                            wqk_sb[:, c * 1024 + cc * 128:
                                   c * 1024 + (cc + 1) * 128],
                            xTb[:, c * TB:(c + 1) * TB],
                            start=(c == 0), stop=(c == HC - 1))

                    def qk_rope(P, cc):
                        u = rp.tile([128, TB], f32, tag="u")
                        nc.scalar.copy(u[:], P[:])
                        rot = rp.tile([128, TB], f32, tag="rot")
                        nc.sync.dma_start(rot[0:64, :], u[64:128, :])
                        nc.sync.dma_start(rot[64:128, :], u[0:64, :])
                        m = rp.tile([128, TB], f32, tag="m")
                        nc.vector.tensor_tensor(
                            out=m[:], in0=rot[:], in1=tsin[:, sl], op=MULT)
                        t = rp.tile([128, TB], f32, tag="t")
                        nc.vector.tensor_tensor(
                            out=t[:], in0=u[:], in1=tcos[:, sl], op=MULT)
                        nc.vector.tensor_tensor(
                            out=qkT[cc][:, sl], in0=t[:], in1=m[:], op=ADD)

                    if jb == 0:
                        # consume hid-chunk quarters in DMA-arrival order
                        Pvs = [psP.tile([128, 512], f32, tag="P",
                                        name=f"Pv{t2}")
                               for t2 in range(4)]
                        for cq in range(4):
                            for t2 in range(4):
                                for c in range(4 * cq, 4 * cq + 4):
                                    v_mm(Pvs[t2], t2, c)
                                if cq == 3:
                                    v_evac(Pvs[t2], t2)
                        for half in (0, 1):
                            Ps = [psP.tile([128, TB], f32, tag="P",
                                           name=f"Pqk{half}{i}")
                                  for i in range(4)]
                            for cq in range(4):
                                for i in range(4):
                                    cc = 4 * half + i
                                    for c in range(4 * cq, 4 * cq + 4):
                                        qk_mm(Ps[i], cc, c)
                                    if cq == 3:
                                        qk_rope(Ps[i], cc)
                    else:
                        for t2 in range(TB // 128):
                            Pv = psP.tile([128, 512], f32, tag="P")
                            for c in range(HC):
                                v_mm(Pv, t2, c)
                            v_evac(Pv, t2)
                        for cc in range(8):
                            P = psP.tile([128, TB], f32, tag="P")
                            for c in range(HC):
                                qk_mm(P, cc, c)
                            qk_rope(P, cc)
                    xTb_cur = xTb_nxt if jb + 1 < NTB else None

            # ---- phases 2+3 interleaved: attention (two head-units in
            # flight), then the finished q-block's out-projection ----
            with tc.tile_pool(name="p2a", bufs=6) as p2a, \
                 tc.tile_pool(name="p2s", bufs=4) as p2s, \
                 tc.tile_pool(name="p2r", bufs=2) as p2r, \
                 tc.tile_pool(name="psS", bufs=2, space="PSUM") as psS, \
                 tc.tile_pool(name="psO", bufs=2, space="PSUM") as psO, \
                 tc.tile_pool(name="psR", bufs=2, space="PSUM") as psR:

                def emit_S(h, jb4, kc, tag):
                    """One S^T chunk matmul; causal mask folded in-PSUM via an
                    identity-lhsT matmul over just the masked column range."""
                    qT_h, kT_h = qkT[h], qkT[NH + h]
                    qsl = slice(jb4 * QB, (jb4 + 1) * QB)
                    md = kc - 4 * jb4
                    Sc = psS.tile([128, QB], f32, tag=tag)
                    nc.tensor.matmul(
                        Sc[:], kT_h[:, kc * 128:(kc + 1) * 128], qT_h[:, qsl],
                        start=True, stop=(md < 0))
                    if md >= 0:
                        nc.tensor.matmul(
                            Sc[:], tid[:], tmask[:, md * QB:(md + 1) * QB],
                            start=False, stop=True)
                    return Sc

                def emit_exp_O(h, kc, Sc, O, nkc):
                    A = p2a.tile([128, QB], bf16, tag="A")
                    nc.scalar.activation(A[:], Sc[:], EXP, scale=SCALE)
                    nc.tensor.matmul(
                        O[:],
                        v_all[:, kc * 512 + h * 128:kc * 512 + (h + 1) * 128],
                        A[:],
                        start=(kc == 0), stop=(kc == nkc - 1))
                    return A

                def emit_R(R, Asum, q, nquad):
                    nc.tensor.matmul(R[:], t1s[:], Asum[:],
                                     start=(q == 0), stop=(q == nquad - 1))

                class Unit:
                    def __init__(self, h, jb4, tag):
                        self.h, self.jb4, self.tag = h, jb4, tag
                        self.nkc = 4 * (jb4 + 1)
                        self.O = psO.tile([128, QB], f32, tag="O")
                        self.R = psR.tile([128, QB], f32, tag="R")
                        self.Sc = [emit_S(h, jb4, 0, tag),
                                   emit_S(h, jb4, 1, tag)]
                        self.A0 = None      # first A of current pair
                        self.Asum_pend = None

                    def step(self, kc):
                        """Emit delayed quad-R, exp+O(kc), pair/quad sums on
                        vector, then lookahead S(kc+2) (after exp so the freed
                        psS bank's reuse is clean)."""
                        nquad = self.nkc // 4
                        if kc % 4 == 1 and self.Asum_pend is not None:
                            emit_R(self.R, self.Asum_pend, kc // 4 - 1, nquad)
                            self.Asum_pend = None
                        A = emit_exp_O(self.h, kc, self.Sc.pop(0), self.O,
                                       self.nkc)
                        if kc % 2 == 0:
                            self.A0 = A
                        else:
                            Asum = p2s.tile([128, QB], bf16, tag="Asum")
                            nc.vector.tensor_tensor(
                                out=Asum[:], in0=self.A0[:], in1=A[:], op=ADD)
                            if kc % 4 == 1:
                                self.Ap = Asum      # first pair of the quad
                            else:
                                Asum2 = p2s.tile([128, QB], bf16, tag="Asum")
                                nc.vector.tensor_tensor(
                                    out=Asum2[:], in0=self.Ap[:], in1=Asum[:],
                                    op=ADD)
                                self.Asum_pend = Asum2
                        if kc + 2 < self.nkc:
                            self.Sc.append(
                                emit_S(self.h, self.jb4, kc + 2, self.tag))

                    def finish(self):
                        nquad = self.nkc // 4
                        emit_R(self.R, self.Asum_pend, nquad - 1, nquad)
                        rec = p2r.tile([128, QB], f32, tag="rec")
                        nc.vector.reciprocal_approx_fast(out=rec[:],
                                                         in_=self.R[:])
                        qsl = slice(self.jb4 * QB, (self.jb4 + 1) * QB)
                        nc.vector.tensor_tensor(
                            out=outT[self.h][:, qsl], in0=self.O[:],
                            in1=rec[:], op=MULT)

                for jb4 in range(NQB):
                    nkc = 4 * (jb4 + 1)
                    for h0 in (0, 2):
                        ua = Unit(h0, jb4, "Sa")
                        ub = Unit(h0 + 1, jb4, "Sb")
                        for kc in range(nkc):
                            ua.step(kc)
                            ub.step(kc)
                        ua.finish()
                        ub.finish()

            # ---- phase 3: out projection (partial) ----
            with tc.tile_pool(name="p3", bufs=4) as p3, \
                 tc.tile_pool(name="ps3", bufs=4, space="PSUM") as ps3:
                for tch in range(S // 128):
                    for cb in range(HID // 512):
                        P3 = ps3.tile([128, 512], f32, tag="P3")
                        for h in range(NH):
                            nc.tensor.matmul(
                                P3[:],
                                outT[h][:, tch * 128:(tch + 1) * 128],
                                wo_sb[:, h * HID + cb * 512:
                                      h * HID + (cb + 1) * 512],
                                start=(h == 0), stop=(h == NH - 1))
                        ys = p3.tile([128, 512], bf16, tag="ys")
                        if (tch * 4 + cb) % 2 == 0:
                            nc.vector.tensor_copy(ys[:], P3[:])
                        else:
                            nc.scalar.copy(ys[:], P3[:])
                        nc.sync.dma_start(
                            y[tch * 128:(tch + 1) * 128,
                              cb * 512:(cb + 1) * 512], ys[:])

    nc.compile()
    return nc


def _host_inputs(x, w_qkv, w_out):
    """Build the 8 per-core input maps, pre-arranged into SBUF layouts."""
    import ml_dtypes
    bf16 = ml_dtypes.bfloat16

    # RoPE tables, [d, t] with the rotate-half sign folded into sin.
    inv_freq = 1.0 / (BASE ** (np.arange(0, D, 2, dtype=np.float64) / D))
    pos = np.arange(S, dtype=np.float64)
    freqs = np.outer(inv_freq, pos)           # [64, S]
    cos_h = np.cos(freqs).astype(np.float32)
    sin_h = np.sin(freqs).astype(np.float32)
    cosT = np.concatenate([cos_h, cos_h], 0)  # [128, S]
    sinS = np.concatenate([-sin_h, sin_h], 0)

    # Additive causal masks for the 4 diagonal chunk offsets ([k, q-free]).
    kp = np.arange(128)[:, None]
    qf = np.arange(QB)[None, :]
    maskM = np.concatenate(
        [np.where(qf < 128 * mm + kp, NEG, 0.0) for mm in range(4)],
        axis=1).astype(bf16)

    w3 = np.asarray(w_qkv, np.float32).reshape(HID, 3, H, D)
    wo_full = np.asarray(w_out, np.float32).reshape(H, D, HID)
    x = np.asarray(x, np.float32)

    shared = {
        "cosT": cosT, "sinS": sinS, "maskM": maskM,
        "identM": np.eye(128, dtype=bf16),
        "ones_sq": np.ones((128, 128), bf16),
    }
    in_maps = []
    for c in range(N_CORES):
        b, hg = c // 4, c % 4
        heads = slice(4 * hg, 4 * hg + 4)
        # xA[p, jb*HC*TB + c*TB + t] = x[b, jb*TB+t, c*128+p]
        xA = np.ascontiguousarray(
            x[b].reshape(NTB, TB, HC, 128).transpose(3, 0, 2, 1)
            .reshape(128, HC * S)).astype(bf16)
        # wqkA[p, c*1024 + cc*128 + j]: cc<4 q heads, cc>=4 k heads
        wqk = w3[:, 0:2, heads, :].reshape(HC, 128, 2 * NH * 128)
        wqkA = np.ascontiguousarray(
            wqk.transpose(1, 0, 2).reshape(128, HC * 1024)).astype(bf16)
        # wvA[p, c*512 + hl*128 + j]
        wv = w3[:, 2, heads, :].reshape(HC, 128, NH * 128)
        wvA = np.ascontiguousarray(
            wv.transpose(1, 0, 2).reshape(128, HC * 512)).astype(bf16)
        # woA[p, hl*HID + n] = wo_full[4hg+hl, p, n]
        woA = np.ascontiguousarray(
            wo_full[heads].transpose(1, 0, 2).reshape(128, NH * HID)
        ).astype(bf16)
        in_maps.append({
            "xA": xA, "wqkA": wqkA, "wvA": wvA, "woA": woA, **shared,
        })
    return in_maps


def kernel(x, w_qkv, w_out):
    from concourse.bass_utils import run_bass_kernel_spmd

    if "nc" not in _cache:
        _cache["nc"] = _build()
    nc = _cache["nc"]
    in_maps = _host_inputs(x, w_qkv, w_out)
    res = run_bass_kernel_spmd(nc, in_maps, core_ids=list(range(N_CORES)))
    out = np.zeros((B, S, HID), np.float32)
    for c in range(N_CORES):
        out[c // 4] += res.results[c]["y"].astype(np.float32)
    return out


# revision 40
# speedup vs baseline: 1.0353x; 1.0009x over previous
"""Multi-head causal attention (B=2, S=2048, HID=2048, H=16, D=128) on 8 TRN2
NeuronCores.

Sharding: core c handles batch b=c//4 and heads [4*(c%4) .. 4*(c%4)+3].
Each core computes qkv-projection + RoPE + causal attention + its partial
out-projection; the host sums the 4 partial outputs per batch (tensor-parallel
reduce) and stacks the 2 batches.

All matmul operands are bf16 (PSUM accumulation stays f32); the PE streams
bf16 at full rate and DMA bytes halve. Host pre-arranges every tensor into
its exact SBUF layout so each load is one flat DMA.

Phases (single TileContext, per-engine in-order queues):
  1. merged QKV projection + RoPE: one pass over x per 512-token block;
     V uses x-stationary matmuls ([tok, head*d] layout), Q/K use
     w-stationary ([d, tok]) with RoPE applied during PSUM evacuation.
  2. attention per (head, q-block): S^T pairs -> exp (causal mask folded
     into the S accumulation as an identity-lhsT matmul adding a -1e9
     staircase) -> O/V accumulation; row sums via ones-matmul on
     vector-pre-summed A pairs; software-pipelined so the tensor engine
     never waits on the scalar exp.
  3. out projection: accumulate 4 heads per 128-token chunk, DMA out.
"""
import sys

sys.path.insert(0, '/opt/trn_rl_repo')

import numpy as np

B, S, HID = 2, 2048, 2048
H, D = 16, 128
NH = H // 4          # heads per core = 4
HC = HID // 128      # hid chunks = 16
TB = 512             # token block for projection
NTB = S // TB        # 4
QB = 512             # q block in attention
NQB = S // QB        # 4
SCALE = 1.0 / float(np.sqrt(D))
BASE = 10000.0
NEG = -1.0e9
N_CORES = 8

_cache = {}


def _build():
    import concourse.bass as bass  # noqa: F401
    import concourse.tile as tile
    from concourse import bacc, mybir

    f32 = mybir.dt.float32
    bf16 = mybir.dt.bfloat16
    EXP = mybir.ActivationFunctionType.Exp
    MULT = mybir.AluOpType.mult
    ADD = mybir.AluOpType.add

    nc = bacc.Bacc("TRN2", target_bir_lowering=False, debug=False,
                   num_devices=N_CORES)

    xA = nc.dram_tensor("xA", [128, HC * S], bf16, kind="ExternalInput").ap()
    # xA layout is per-jb-block contiguous: [p, jb*HC*TB + c*TB + t]
    wqkA = nc.dram_tensor("wqkA", [128, HC * 8 * 128], bf16,
                          kind="ExternalInput").ap()
    wvA = nc.dram_tensor("wvA", [128, HC * NH * 128], bf16,
                         kind="ExternalInput").ap()
    woA = nc.dram_tensor("woA", [128, NH * HID], bf16,
                         kind="ExternalInput").ap()
    cosT = nc.dram_tensor("cosT", [128, S], f32, kind="ExternalInput").ap()
    sinS = nc.dram_tensor("sinS", [128, S], f32, kind="ExternalInput").ap()
    maskM = nc.dram_tensor("maskM", [128, 4 * QB], bf16,
                           kind="ExternalInput").ap()
    identM = nc.dram_tensor("identM", [128, 128], bf16,
                            kind="ExternalInput").ap()
    ones_sq = nc.dram_tensor("ones_sq", [128, 128], bf16,
                             kind="ExternalInput").ap()
    y = nc.dram_tensor("y", [S, HID], bf16, kind="ExternalOutput").ap()

    with tile.TileContext(nc) as tc:
        with tc.tile_pool(name="persist", bufs=1) as pp:
            # ---- resident tensors ----
            xTb_n = HC * TB
            wqk_sb = pp.tile([128, HC * 1024], bf16, tag="wqk_sb")
            wv_sb = pp.tile([128, HC * 512], bf16, tag="wv_sb")
            wo_sb = pp.tile([128, NH * HID], bf16, tag="wo_sb")
            tcos = pp.tile([128, S], f32, tag="tcos")
            tsin = pp.tile([128, S], f32, tag="tsin")
            tmask = pp.tile([128, 4 * QB], bf16, tag="tmask")
            tid = pp.tile([128, 128], bf16, tag="tid")
            t1s = pp.tile([128, 128], bf16, tag="t1s")
            qkT = [pp.tile([128, S], bf16, tag=f"qkT{i}", name=f"qkT{i}")
                   for i in range(8)]
            v_all = pp.tile([128, HC * NH * 128], bf16, tag="v_all")
            outT = [pp.tile([128, S], bf16, tag=f"outT{h}", name=f"outT{h}")
                    for h in range(NH)]

            # ---- phase 1: merged V + QK projection with RoPE ----
            with tc.tile_pool(name="p1x", bufs=2) as p1x, \
                 tc.tile_pool(name="rope", bufs=3) as rp, \
                 tc.tile_pool(name="ropeu", bufs=4) as rpu, \
                 tc.tile_pool(name="psP", bufs=6, space="PSUM") as psP:
                # issue order = first-use order; quartered so the first
                # accumulation groups start on the first arrivals
                xTb0 = p1x.tile([128, xTb_n], bf16, tag="xTb")
                for sp in range(8):
                    xs = slice(sp * 2 * TB, (sp + 1) * 2 * TB)
                    ws = slice(sp * 2 * 512, (sp + 1) * 2 * 512)
                    nc.sync.dma_start(xTb0[:, xs], xA[:, xs])
                    nc.sync.dma_start(wv_sb[:, ws], wvA[:, ws])
                for sp in range(4):
                    qs = slice(sp * 4 * 1024, (sp + 1) * 4 * 1024)
                    nc.sync.dma_start(wqk_sb[:, qs], wqkA[:, qs])
                nc.sync.dma_start(tcos[:], cosT[:])
                nc.sync.dma_start(tsin[:], sinS[:])

                xTb_cur = xTb0
                for jb in range(NTB):
                    # prefetch next x block
                    if jb + 1 < NTB:
                        xTb_nxt = p1x.tile([128, xTb_n], bf16, tag="xTb")
                        nc.sync.dma_start(
                            xTb_nxt[:],
                            xA[:, (jb + 1) * xTb_n:(jb + 2) * xTb_n])
                    if jb == 1:
                        # later-phase loads, queued after the jb prefetches
                        nc.sync.dma_start(tmask[:], maskM[:])
                        nc.sync.dma_start(tid[:], identM[:])
                        nc.sync.dma_start(t1s[:], ones_sq[:])
                        nc.sync.dma_start(wo_sb[:], woA[:])
                    xTb = xTb_cur
                    sl = slice(jb * TB, (jb + 1) * TB)

                    def v_mm(Pv, t2, c):
                        nc.tensor.matmul(
                            Pv[:],
                            xTb[:, c * TB + t2 * 128:c * TB + (t2 + 1) * 128],
                            wv_sb[:, c * 512:(c + 1) * 512],
                            start=(c == 0), stop=(c == HC - 1))

                    def v_evac(Pv, t2):
                        cg = jb * (TB // 128) + t2
                        nc.scalar.copy(v_all[:, cg * 512:(cg + 1) * 512],
                                       Pv[:])

                    def qk_mm(P, cc, c):
                        nc.tensor.matmul(
                            P[:],
                            wqk_sb[:, c * 1024 + cc * 128:
                                   c * 1024 + (cc + 1) * 128],
                            xTb[:, c * TB:(c + 1) * TB],
                            start=(c == 0), stop=(c == HC - 1))

                    def qk_rope(P, cc):
                        u = rpu.tile([128, TB], f32, tag="u")
                        nc.scalar.copy(u[:], P[:])
                        rot = rp.tile([128, TB], f32, tag="rot")
                        nc.sync.dma_start(rot[0:64, :], u[64:128, :])
                        nc.sync.dma_start(rot[64:128, :], u[0:64, :])
                        m = rp.tile([128, TB], f32, tag="m")
                        nc.vector.tensor_tensor(
                            out=m[:], in0=rot[:], in1=tsin[:, sl], op=MULT)
                        t = rp.tile([128, TB], f32, tag="t")
                        nc.vector.tensor_tensor(
                            out=t[:], in0=u[:], in1=tcos[:, sl], op=MULT)
                        nc.vector.tensor_tensor(
                            out=qkT[cc][:, sl], in0=t[:], in1=m[:], op=ADD)

                    if jb == 0:
                        # consume hid-chunk quarters in DMA-arrival order
                        Pvs = [psP.tile([128, 512], f32, tag="P",
                                        name=f"Pv{t2}")
                               for t2 in range(4)]
                        for cq in range(4):
                            for t2 in range(4):
                                for c in range(4 * cq, 4 * cq + 4):
                                    v_mm(Pvs[t2], t2, c)
                                if cq == 3:
                                    v_evac(Pvs[t2], t2)
                        for half in (0, 1):
                            Ps = [psP.tile([128, TB], f32, tag="P",
                                           name=f"Pqk{half}{i}")
                                  for i in range(4)]
                            for cq in range(4):
                                for i in range(4):
                                    cc = 4 * half + i
                                    for c in range(4 * cq, 4 * cq + 4):
                                        qk_mm(Ps[i], cc, c)
                                    if cq == 3:
                                        qk_rope(Ps[i], cc)
                    else:
                        for t2 in range(TB // 128):
                            Pv = psP.tile([128, 512], f32, tag="P")
                            for c in range(HC):
                                v_mm(Pv, t2, c)
                            v_evac(Pv, t2)
                        for cc in range(8):
                            P = psP.tile([128, TB], f32, tag="P")
                            for c in range(HC):
                                qk_mm(P, cc, c)
                            qk_rope(P, cc)
                    xTb_cur = xTb_nxt if jb + 1 < NTB else None

            # ---- phases 2+3 interleaved: attention (two head-units in
            # flight), then the finished q-block's out-projection ----
            with tc.tile_pool(name="p2a", bufs=6) as p2a, \
                 tc.tile_pool(name="p2s", bufs=4) as p2s, \
                 tc.tile_pool(name="p2r", bufs=2) as p2r, \
                 tc.tile_pool(name="psS", bufs=2, space="PSUM") as psS, \
                 tc.tile_pool(name="psO", bufs=2, space="PSUM") as psO, \
                 tc.tile_pool(name="psR", bufs=2, space="PSUM") as psR:

                def emit_S(h, jb4, kc, tag):
                    """One S^T chunk matmul; causal mask folded in-PSUM via an
                    identity-lhsT matmul over just the masked column range."""
                    qT_h, kT_h = qkT[h], qkT[NH + h]
                    qsl = slice(jb4 * QB, (jb4 + 1) * QB)
                    md = kc - 4 * jb4
                    Sc = psS.tile([128, QB], f32, tag=tag)
                    nc.tensor.matmul(
                        Sc[:], kT_h[:, kc * 128:(kc + 1) * 128], qT_h[:, qsl],
                        start=True, stop=(md < 0))
                    if md >= 0:
                        nc.tensor.matmul(
                            Sc[:], tid[:], tmask[:, md * QB:(md + 1) * QB],
                            start=False, stop=True)
                    return Sc

                def emit_exp_O(h, kc, Sc, O, nkc):
                    A = p2a.tile([128, QB], bf16, tag="A")
                    nc.scalar.activation(A[:], Sc[:], EXP, scale=SCALE)
                    nc.tensor.matmul(
                        O[:],
                        v_all[:, kc * 512 + h * 128:kc * 512 + (h + 1) * 128],
                        A[:],
                        start=(kc == 0), stop=(kc == nkc - 1))
                    return A

                def emit_R(R, Asum, q, nquad):
                    nc.tensor.matmul(R[:], t1s[:], Asum[:],
                                     start=(q == 0), stop=(q == nquad - 1))

                class Unit:
                    def __init__(self, h, jb4, tag):
                        self.h, self.jb4, self.tag = h, jb4, tag
                        self.nkc = 4 * (jb4 + 1)
                        self.O = psO.tile([128, QB], f32, tag="O")
                        self.R = psR.tile([128, QB], f32, tag="R")
                        self.Sc = [emit_S(h, jb4, 0, tag),
                                   emit_S(h, jb4, 1, tag)]
                        self.A0 = None      # first A of current pair
                        self.Asum_pend = None

                    def step(self, kc):
                        """Emit delayed quad-R, exp+O(kc), pair/quad sums on
                        vector, then lookahead S(kc+2) (after exp so the freed
                        psS bank's reuse is clean)."""
                        nquad = self.nkc // 4
                        if kc % 4 == 1 and self.Asum_pend is not None:
                            emit_R(self.R, self.Asum_pend, kc // 4 - 1, nquad)
                            self.Asum_pend = None
                        A = emit_exp_O(self.h, kc, self.Sc.pop(0), self.O,
                                       self.nkc)
                        if kc % 2 == 0:
                            self.A0 = A
                        else:
                            Asum = p2s.tile([128, QB], bf16, tag="Asum")
                            nc.vector.tensor_tensor(
                                out=Asum[:], in0=self.A0[:], in1=A[:], op=ADD)
                            if kc % 4 == 1:
                                self.Ap = Asum      # first pair of the quad
                            else:
                                Asum2 = p2s.tile([128, QB], bf16, tag="Asum")
                                nc.vector.tensor_tensor(
                                    out=Asum2[:], in0=self.Ap[:], in1=Asum[:],
                                    op=ADD)
                                self.Asum_pend = Asum2
                        if kc + 2 < self.nkc:
                            self.Sc.append(
                                emit_S(self.h, self.jb4, kc + 2, self.tag))

                    def finish(self):
                        nquad = self.nkc // 4
                        emit_R(self.R, self.Asum_pend, nquad - 1, nquad)
                        rec = p2r.tile([128, QB], f32, tag="rec")
                        nc.vector.reciprocal_approx_fast(out=rec[:],
                                                         in_=self.R[:])
                        qsl = slice(self.jb4 * QB, (self.jb4 + 1) * QB)
                        nc.vector.tensor_tensor(
                            out=outT[self.h][:, qsl], in0=self.O[:],
                            in1=rec[:], op=MULT)

                for jb4 in range(NQB):
                    nkc = 4 * (jb4 + 1)
                    for h0 in (0, 2):
                        ua = Unit(h0, jb4, "Sa")
                        ub = Unit(h0 + 1, jb4, "Sb")
                        for kc in range(nkc):
                            ua.step(kc)
                            ub.step(kc)
                        ua.finish()
                        ub.finish()

            # ---- phase 3: out projection (partial) ----
            with tc.tile_pool(name="p3", bufs=4) as p3, \
                 tc.tile_pool(name="ps3", bufs=4, space="PSUM") as ps3:
                for tch in range(S // 128):
                    for cb in range(HID // 512):
                        P3 = ps3.tile([128, 512], f32, tag="P3")
                        for h in range(NH):
                            nc.tensor.matmul(
                                P3[:],
                                outT[h][:, tch * 128:(tch + 1) * 128],
                                wo_sb[:, h * HID + cb * 512:
                                      h * HID + (cb + 1) * 512],
                                start=(h == 0), stop=(h == NH - 1))
                        ys = p3.tile([128, 512], bf16, tag="ys")
                        if (tch * 4 + cb) % 2 == 0:
                            nc.vector.tensor_copy(ys[:], P3[:])
                        else:
                            nc.scalar.copy(ys[:], P3[:])
                        nc.sync.dma_start(
                            y[tch * 128:(tch + 1) * 128,
                              cb * 512:(cb + 1) * 512], ys[:])

    nc.compile()
    return nc


def _host_inputs(x, w_qkv, w_out):
    """Build the 8 per-core input maps, pre-arranged into SBUF layouts."""
    import ml_dtypes
    bf16 = ml_dtypes.bfloat16

    # RoPE tables, [d, t] with the rotate-half sign folded into sin.
    inv_freq = 1.0 / (BASE ** (np.arange(0, D, 2, dtype=np.float64) / D))
    pos = np.arange(S, dtype=np.float64)
    freqs = np.outer(inv_freq, pos)           # [64, S]
    cos_h = np.cos(freqs).astype(np.float32)
    sin_h = np.sin(freqs).astype(np.float32)
    cosT = np.concatenate([cos_h, cos_h], 0)  # [128, S]
    sinS = np.concatenate([-sin_h, sin_h], 0)

    # Additive causal masks for the 4 diagonal chunk offsets ([k, q-free]).
    kp = np.arange(128)[:, None]
    qf = np.arange(QB)[None, :]
    maskM = np.concatenate(
        [np.where(qf < 128 * mm + kp, NEG, 0.0) for mm in range(4)],
        axis=1).astype(bf16)

    w3 = np.asarray(w_qkv, np.float32).reshape(HID, 3, H, D)
    wo_full = np.asarray(w_out, np.float32).reshape(H, D, HID)
    x = np.asarray(x, np.float32)

    shared = {
        "cosT": cosT, "sinS": sinS, "maskM": maskM,
        "identM": np.eye(128, dtype=bf16),
        "ones_sq": np.ones((128, 128), bf16),
    }
    in_maps = []
    for c in range(N_CORES):
        b, hg = c // 4, c % 4
        heads = slice(4 * hg, 4 * hg + 4)
        # xA[p, jb*HC*TB + c*TB + t] = x[b, jb*TB+t, c*128+p]
        xA = np.ascontiguousarray(
            x[b].reshape(NTB, TB, HC, 128).transpose(3, 0, 2, 1)
            .reshape(128, HC * S)).astype(bf16)
        # wqkA[p, c*1024 + cc*128 + j]: cc<4 q heads, cc>=4 k heads
        wqk = w3[:, 0:2, heads, :].reshape(HC, 128, 2 * NH * 128)
        wqkA = np.ascontiguousarray(
            wqk.transpose(1, 0, 2).reshape(128, HC * 1024)).astype(bf16)
        # wvA[p, c*512 + hl*128 + j]
        wv = w3[:, 2, heads, :].reshape(HC, 128, NH * 128)
        wvA = np.ascontiguousarray(
            wv.transpose(1, 0, 2).reshape(128, HC * 512)).astype(bf16)
        # woA[p, hl*HID + n] = wo_full[4hg+hl, p, n]
        woA = np.ascontiguousarray(
            wo_full[heads].transpose(1, 0, 2).reshape(128, NH * HID)
        ).astype(bf16)
        in_maps.append({
            "xA": xA, "wqkA": wqkA, "wvA": wvA, "woA": woA, **shared,
        })
    return in_maps


def kernel(x, w_qkv, w_out):
    from concourse.bass_utils import run_bass_kernel_spmd

    if "nc" not in _cache:
        _cache["nc"] = _build()
    nc = _cache["nc"]
    in_maps = _host_inputs(x, w_qkv, w_out)
    res = run_bass_kernel_spmd(nc, in_maps, core_ids=list(range(N_CORES)))
    out = np.zeros((B, S, HID), np.float32)
    for c in range(N_CORES):
        out[c // 4] += res.results[c]["y"].astype(np.float32)
    return out


# revision 44
# speedup vs baseline: 1.0661x; 1.0298x over previous
"""Multi-head causal attention (B=2, S=2048, HID=2048, H=16, D=128) on 8 TRN2
NeuronCores.

Sharding: core c handles batch b=c//4 and heads [4*(c%4) .. 4*(c%4)+3].
Each core computes qkv-projection + RoPE + causal attention + its partial
out-projection; the host sums the 4 partial outputs per batch (tensor-parallel
reduce) and stacks the 2 batches.

All matmul operands are bf16 (PSUM accumulation stays f32); the PE streams
bf16 at full rate and DMA bytes halve. Host pre-arranges every tensor into
its exact SBUF layout so each load is one flat DMA.

Phases (single TileContext, per-engine in-order queues):
  1. merged QKV projection + RoPE: one pass over x per 512-token block;
     V uses x-stationary matmuls ([tok, head*d] layout), Q/K use
     w-stationary ([d, tok]) with RoPE applied during PSUM evacuation.
     The first block's groups consume hid-chunk quarters in DMA-arrival
     order so compute starts on the first eighth of the startup loads.
  2. attention with TWO head-units software-pipelined in lockstep (each
     per-chunk: S^T matmul -> causal mask folded in-PSUM via an
     identity-lhsT matmul adding a -1e9 staircase -> scalar exp -> O/V
     accumulation). Interleaving two units gives the exp ~2 matmul-slots
     of latency cover, so the tensor engine never idles; softmax row
     sums come from a ones-matmul per vector-pre-summed chunk QUAD, and
     normalization uses reciprocal_approx_fast.
  3. out projection: accumulate 4 heads per 128-token chunk, DMA out
     (bf16 partials; the host upcasts and reduces).
"""
import sys

sys.path.insert(0, '/opt/trn_rl_repo')

import numpy as np

B, S, HID = 2, 2048, 2048
H, D = 16, 128
NH = H // 4          # heads per core = 4
HC = HID // 128      # hid chunks = 16
TB = 512             # token block for projection
NTB = S // TB        # 4
QB = 512             # q block in attention
NQB = S // QB        # 4
SCALE = 1.0 / float(np.sqrt(D))
BASE = 10000.0
NEG = -1.0e9
N_CORES = 8

_cache = {}


def _build():
    import concourse.bass as bass  # noqa: F401
    import concourse.tile as tile
    from concourse import bacc, mybir

    f32 = mybir.dt.float32
    bf16 = mybir.dt.bfloat16
    EXP = mybir.ActivationFunctionType.Exp
    MULT = mybir.AluOpType.mult
    ADD = mybir.AluOpType.add

    nc = bacc.Bacc("TRN2", target_bir_lowering=False, debug=False,
                   num_devices=N_CORES)

    xA = nc.dram_tensor("xA", [128, HC * S], bf16, kind="ExternalInput").ap()
    # xA layout is per-jb-block contiguous: [p, jb*HC*TB + c*TB + t]
    wqkA = nc.dram_tensor("wqkA", [128, HC * 8 * 128], bf16,
                          kind="ExternalInput").ap()
    wvA = nc.dram_tensor("wvA", [128, HC * NH * 128], bf16,
                         kind="ExternalInput").ap()
    woA = nc.dram_tensor("woA", [128, NH * HID], bf16,
                         kind="ExternalInput").ap()
    cosT = nc.dram_tensor("cosT", [128, S], f32, kind="ExternalInput").ap()
    sinS = nc.dram_tensor("sinS", [128, S], f32, kind="ExternalInput").ap()
    maskM = nc.dram_tensor("maskM", [128, 4 * QB], bf16,
                           kind="ExternalInput").ap()
    identM = nc.dram_tensor("identM", [128, 128], bf16,
                            kind="ExternalInput").ap()
    ones_sq = nc.dram_tensor("ones_sq", [128, 128], bf16,
                             kind="ExternalInput").ap()
    y = nc.dram_tensor("y", [S, HID], bf16, kind="ExternalOutput").ap()

    with tile.TileContext(nc) as tc:
        with tc.tile_pool(name="persist", bufs=1) as pp:
            # ---- resident tensors ----
            xTb_n = HC * TB
            wqk_sb = pp.tile([128, HC * 1024], bf16, tag="wqk_sb")
            wv_sb = pp.tile([128, HC * 512], bf16, tag="wv_sb")
            wo_sb = pp.tile([128, NH * HID], bf16, tag="wo_sb")
            tcos = pp.tile([128, S], f32, tag="tcos")
            tsin = pp.tile([128, S], f32, tag="tsin")
            tmask = pp.tile([128, 4 * QB], bf16, tag="tmask")
            tid = pp.tile([128, 128], bf16, tag="tid")
            t1s = pp.tile([128, 128], bf16, tag="t1s")
            qkT = [pp.tile([128, S], bf16, tag=f"qkT{i}", name=f"qkT{i}")
                   for i in range(8)]
            v_all = pp.tile([128, HC * NH * 128], bf16, tag="v_all")
            outT = [pp.tile([128, S], bf16, tag=f"outT{h}", name=f"outT{h}")
                    for h in range(NH)]

            # ---- phase 1: merged V + QK projection with RoPE ----
            with tc.tile_pool(name="p1x", bufs=2) as p1x, \
                 tc.tile_pool(name="rope", bufs=3) as rp, \
                 tc.tile_pool(name="ropeu", bufs=4) as rpu, \
                 tc.tile_pool(name="psP", bufs=6, space="PSUM") as psP:
                # issue order = first-use order; quartered so the first
                # accumulation groups start on the first arrivals
                xTb0 = p1x.tile([128, xTb_n], bf16, tag="xTb")
                for sp in range(8):
                    xs = slice(sp * 2 * TB, (sp + 1) * 2 * TB)
                    ws = slice(sp * 2 * 512, (sp + 1) * 2 * 512)
                    nc.sync.dma_start(xTb0[:, xs], xA[:, xs])
                    nc.sync.dma_start(wv_sb[:, ws], wvA[:, ws])
                for sp in range(4):
                    qs = slice(sp * 4 * 1024, (sp + 1) * 4 * 1024)
                    nc.sync.dma_start(wqk_sb[:, qs], wqkA[:, qs])
                nc.sync.dma_start(tcos[:], cosT[:])
                nc.sync.dma_start(tsin[:], sinS[:])

                xTb_cur = xTb0
                for jb in range(NTB):
                    # prefetch next x block
                    if jb + 1 < NTB:
                        xTb_nxt = p1x.tile([128, xTb_n], bf16, tag="xTb")
                        nc.sync.dma_start(
                            xTb_nxt[:],
                            xA[:, (jb + 1) * xTb_n:(jb + 2) * xTb_n])
                    if jb == 1:
                        # later-phase loads, queued after the jb prefetches
                        nc.sync.dma_start(tmask[:], maskM[:])
                        nc.sync.dma_start(tid[:], identM[:])
                        nc.sync.dma_start(t1s[:], ones_sq[:])
                        nc.sync.dma_start(wo_sb[:], woA[:])
                    xTb = xTb_cur
                    sl = slice(jb * TB, (jb + 1) * TB)

                    def v_mm(Pv, t2, c):
                        nc.tensor.matmul(
                            Pv[:],
                            xTb[:, c * TB + t2 * 128:c * TB + (t2 + 1) * 128],
                            wv_sb[:, c * 512:(c + 1) * 512],
                            start=(c == 0), stop=(c == HC - 1))

                    def v_evac(Pv, t2):
                        cg = jb * (TB // 128) + t2
                        nc.scalar.copy(v_all[:, cg * 512:(cg + 1) * 512],
                                       Pv[:])

                    def qk_mm(P, cc, c):
                        nc.tensor.matmul(
                            P[:],
                            wqk_sb[:, c * 1024 + cc * 128:
                                   c * 1024 + (cc + 1) * 128],
                            xTb[:, c * TB:(c + 1) * TB],
                            start=(c == 0), stop=(c == HC - 1))

                    def qk_rope(P, cc):
                        u = rpu.tile([128, TB], f32, tag="u")
                        nc.scalar.copy(u[:], P[:])
                        rot = rp.tile([128, TB], f32, tag="rot")
                        nc.sync.dma_start(rot[0:64, :], u[64:128, :])
                        nc.sync.dma_start(rot[64:128, :], u[0:64, :])
                        m = rp.tile([128, TB], f32, tag="m")
                        nc.vector.tensor_tensor(
                            out=m[:], in0=rot[:], in1=tsin[:, sl], op=MULT)
                        t = rp.tile([128, TB], f32, tag="t")
                        nc.vector.tensor_tensor(
                            out=t[:], in0=u[:], in1=tcos[:, sl], op=MULT)
                        nc.vector.tensor_tensor(
                            out=qkT[cc][:, sl], in0=t[:], in1=m[:], op=ADD)

                    if jb == 0:
                        # consume hid-chunk quarters in DMA-arrival order
                        Pvs = [psP.tile([128, 512], f32, tag="P",
                                        name=f"Pv{t2}")
                               for t2 in range(4)]
                        for cq in range(4):
                            for t2 in range(4):
                                for c in range(4 * cq, 4 * cq + 4):
                                    v_mm(Pvs[t2], t2, c)
                                if cq == 3:
                                    v_evac(Pvs[t2], t2)
                        for half in (0, 1):
                            Ps = [psP.tile([128, TB], f32, tag="P",
                                           name=f"Pqk{half}{i}")
                                  for i in range(4)]
                            for cq in range(4):
                                for i in range(4):
                                    cc = 4 * half + i
                                    for c in range(4 * cq, 4 * cq + 4):
                                        qk_mm(Ps[i], cc, c)
                                    if cq == 3:
                                        qk_rope(Ps[i], cc)
                    else:
                        for t2 in range(TB // 128):
                            Pv = psP.tile([128, 512], f32, tag="P")
                            for c in range(HC):
                                v_mm(Pv, t2, c)
                            v_evac(Pv, t2)
                        for cc in range(8):
                            P = psP.tile([128, TB], f32, tag="P")
                            for c in range(HC):
                                qk_mm(P, cc, c)
                            qk_rope(P, cc)
                    xTb_cur = xTb_nxt if jb + 1 < NTB else None

            # ---- phases 2+3 interleaved: attention (two head-units in
            # flight), then the finished q-block's out-projection ----
            with tc.tile_pool(name="p2a", bufs=6) as p2a, \
                 tc.tile_pool(name="p2s", bufs=4) as p2s, \
                 tc.tile_pool(name="p2r", bufs=2) as p2r, \
                 tc.tile_pool(name="psS", bufs=2, space="PSUM") as psS, \
                 tc.tile_pool(name="psO", bufs=2, space="PSUM") as psO, \
                 tc.tile_pool(name="psR", bufs=2, space="PSUM") as psR:

                def trim_off(jb4, md):
                    """Columns [0, off) of a diagonal chunk's q-strip are
                    fully masked; skip them (jb4>0: last two chunks)."""
                    return 256 if (jb4 > 0 and md >= 2) else 0

                def emit_S(h, jb4, kc, tag):
                    """One S^T chunk matmul; causal mask folded in-PSUM via an
                    identity-lhsT matmul over just the staircase extent."""
                    qT_h, kT_h = qkT[h], qkT[NH + h]
                    md = kc - 4 * jb4
                    off = trim_off(jb4, md)
                    qsl = slice(jb4 * QB + off, (jb4 + 1) * QB)
                    Sc = psS.tile([128, QB], f32, tag=tag)
                    nc.tensor.matmul(
                        Sc[:, off:QB], kT_h[:, kc * 128:(kc + 1) * 128],
                        qT_h[:, qsl],
                        start=True, stop=(md < 0))
                    if md >= 0:
                        mw = min(128 * (md + 1), QB)
                        nc.tensor.matmul(
                            Sc[:, off:mw], tid[:],
                            tmask[:, md * QB + off:md * QB + mw],
                            start=False, stop=True,
                            skip_group_check=(mw < QB))
                    return Sc

                def emit_exp_O(h, kc, Sc, O, nkc, jb4, first, last):
                    md = kc - 4 * jb4
                    off = trim_off(jb4, md)
                    A = p2a.tile([128, QB], bf16, tag="A")
                    nc.scalar.activation(A[:, off:QB], Sc[:, off:QB], EXP,
                                         scale=SCALE)
                    nc.tensor.matmul(
                        O[:, off:QB],
                        v_all[:, kc * 512 + h * 128:kc * 512 + (h + 1) * 128],
                        A[:, off:QB],
                        start=first, stop=last,
                        skip_group_check=(off > 0))
                    return A

                def emit_R(R, Asum, q, nquad):
                    nc.tensor.matmul(R[:], t1s[:], Asum[:],
                                     start=(q == 0), stop=(q == nquad - 1))

                class Unit:
                    def __init__(self, h, jb4, tag):
                        self.h, self.jb4, self.tag = h, jb4, tag
                        nkc = self.nkc = 4 * (jb4 + 1)
                        # process trimmed diagonal chunks before md0/md1 so
                        # the O group's start AND stop land on full widths
                        if jb4 == 0:
                            self.proc = [0, 1, 2, 3]
                        else:
                            self.proc = (list(range(4 * jb4)) +
                                         [nkc - 2, nkc - 1, nkc - 4, nkc - 3])
                        self.O = psO.tile([128, QB], f32, tag="O")
                        self.R = psR.tile([128, QB], f32, tag="R")
                        self.Sc = [emit_S(h, jb4, self.proc[0], tag),
                                   emit_S(h, jb4, self.proc[1], tag)]
                        self.A0 = None      # first A of current pair
                        self.Ap = None      # first pair-sum of current quad
                        self.offp = 0
                        self.Asum_pend = None

                    def step(self, i):
                        """Emit delayed quad-R, exp+O, pair/quad sums on
                        vector, then lookahead S (after exp so the freed
                        psS bank's reuse is clean)."""
                        nquad = self.nkc // 4
                        if i % 4 == 1 and self.Asum_pend is not None:
                            emit_R(self.R, self.Asum_pend, i // 4 - 1, nquad)
                            self.Asum_pend = None
                        kc = self.proc[i]
                        A = emit_exp_O(self.h, kc, self.Sc.pop(0), self.O,
                                       self.nkc, self.jb4,
                                       first=(i == 0),
                                       last=(i == self.nkc - 1))
                        off = trim_off(self.jb4, kc - 4 * self.jb4)
                        if i % 2 == 0:
                            self.A0 = A
                        else:
                            # both members of a pair share the same trim
                            Asum = p2s.tile([128, QB], bf16, tag="Asum")
                            nc.vector.tensor_tensor(
                                out=Asum[:, off:QB], in0=self.A0[:, off:QB],
                                in1=A[:, off:QB], op=ADD)
                            if i % 4 == 1:
                                self.Ap, self.offp = Asum, off
                            elif off == 0 and self.offp == 0:
                                Asum2 = p2s.tile([128, QB], bf16, tag="Asum")
                                nc.vector.tensor_tensor(
                                    out=Asum2[:], in0=self.Ap[:], in1=Asum[:],
                                    op=ADD)
                                self.Asum_pend = Asum2
                            else:
                                # one pair is a trimmed strip: fold it into
                                # the full pair's sum in place
                                full, part, poff = (
                                    (self.Ap, Asum, off) if self.offp == 0
                                    else (Asum, self.Ap, self.offp))
                                nc.vector.tensor_tensor(
                                    out=full[:, poff:QB],
                                    in0=full[:, poff:QB],
                                    in1=part[:, poff:QB], op=ADD)
                                self.Asum_pend = full
                        if i + 2 < self.nkc:
                            self.Sc.append(
                                emit_S(self.h, self.jb4, self.proc[i + 2],
                                       self.tag))

                    def finish(self):
                        nquad = self.nkc // 4
                        emit_R(self.R, self.Asum_pend, nquad - 1, nquad)
                        rec = p2r.tile([128, QB], f32, tag="rec")
                        nc.vector.reciprocal_approx_fast(out=rec[:],
                                                         in_=self.R[:])
                        qsl = slice(self.jb4 * QB, (self.jb4 + 1) * QB)
                        nc.vector.tensor_tensor(
                            out=outT[self.h][:, qsl], in0=self.O[:],
                            in1=rec[:], op=MULT)

                for jb4 in range(NQB):
                    nkc = 4 * (jb4 + 1)
                    for h0 in (0, 2):
                        ua = Unit(h0, jb4, "Sa")
                        ub = Unit(h0 + 1, jb4, "Sb")
                        for i in range(nkc):
                            ua.step(i)
                            ub.step(i)
                        ua.finish()
                        ub.finish()

            # ---- phase 3: out projection (partial) ----
            with tc.tile_pool(name="p3", bufs=4) as p3, \
                 tc.tile_pool(name="ps3", bufs=4, space="PSUM") as ps3:
                for tch in range(S // 128):
                    for cb in range(HID // 512):
                        P3 = ps3.tile([128, 512], f32, tag="P3")
                        for h in range(NH):
                            nc.tensor.matmul(
                                P3[:],
                                outT[h][:, tch * 128:(tch + 1) * 128],
                                wo_sb[:, h * HID + cb * 512:
                                      h * HID + (cb + 1) * 512],
                                start=(h == 0), stop=(h == NH - 1))
                        ys = p3.tile([128, 512], bf16, tag="ys")
                        if (tch * 4 + cb) % 2 == 0:
                            nc.vector.tensor_copy(ys[:], P3[:])
                        else:
                            nc.scalar.copy(ys[:], P3[:])
                        nc.sync.dma_start(
                            y[tch * 128:(tch + 1) * 128,
                              cb * 512:(cb + 1) * 512], ys[:])

    nc.compile()
    return nc


def _host_inputs(x, w_qkv, w_out):
    """Build the 8 per-core input maps, pre-arranged into SBUF layouts."""
    import ml_dtypes
    bf16 = ml_dtypes.bfloat16

    # RoPE tables, [d, t] with the rotate-half sign folded into sin.
    inv_freq = 1.0 / (BASE ** (np.arange(0, D, 2, dtype=np.float64) / D))
    pos = np.arange(S, dtype=np.float64)
    freqs = np.outer(inv_freq, pos)           # [64, S]
    cos_h = np.cos(freqs).astype(np.float32)
    sin_h = np.sin(freqs).astype(np.float32)
    cosT = np.concatenate([cos_h, cos_h], 0)  # [128, S]
    sinS = np.concatenate([-sin_h, sin_h], 0)

    # Additive causal masks for the 4 diagonal chunk offsets ([k, q-free]).
    kp = np.arange(128)[:, None]
    qf = np.arange(QB)[None, :]
    maskM = np.concatenate(
        [np.where(qf < 128 * mm + kp, NEG, 0.0) for mm in range(4)],
        axis=1).astype(bf16)

    w3 = np.asarray(w_qkv, np.float32).reshape(HID, 3, H, D)
    wo_full = np.asarray(w_out, np.float32).reshape(H, D, HID)
    x = np.asarray(x, np.float32)

    shared = {
        "cosT": cosT, "sinS": sinS, "maskM": maskM,
        "identM": np.eye(128, dtype=bf16),
        "ones_sq": np.ones((128, 128), bf16),
    }
    in_maps = []
    for c in range(N_CORES):
        b, hg = c // 4, c % 4
        heads = slice(4 * hg, 4 * hg + 4)
        # xA[p, jb*HC*TB + c*TB + t] = x[b, jb*TB+t, c*128+p]
        xA = np.ascontiguousarray(
            x[b].reshape(NTB, TB, HC, 128).transpose(3, 0, 2, 1)
            .reshape(128, HC * S)).astype(bf16)
        # wqkA[p, c*1024 + cc*128 + j]: cc<4 q heads, cc>=4 k heads
        wqk = w3[:, 0:2, heads, :].reshape(HC, 128, 2 * NH * 128)
        wqkA = np.ascontiguousarray(
            wqk.transpose(1, 0, 2).reshape(128, HC * 1024)).astype(bf16)
        # wvA[p, c*512 + hl*128 + j]
        wv = w3[:, 2, heads, :].reshape(HC, 128, NH * 128)
        wvA = np.ascontiguousarray(
            wv.transpose(1, 0, 2).reshape(128, HC * 512)).astype(bf16)
        # woA[p, hl*HID + n] = wo_full[4hg+hl, p, n]
        woA = np.ascontiguousarray(
            wo_full[heads].transpose(1, 0, 2).reshape(128, NH * HID)
        ).astype(bf16)
        in_maps.append({
            "xA": xA, "wqkA": wqkA, "wvA": wvA, "woA": woA, **shared,
        })
    return in_maps


def kernel(x, w_qkv, w_out):
    from concourse.bass_utils import run_bass_kernel_spmd

    if "nc" not in _cache:
        _cache["nc"] = _build()
    nc = _cache["nc"]
    in_maps = _host_inputs(x, w_qkv, w_out)
    res = run_bass_kernel_spmd(nc, in_maps, core_ids=list(range(N_CORES)))
    out = np.zeros((B, S, HID), np.float32)
    for c in range(N_CORES):
        out[c // 4] += res.results[c]["y"].astype(np.float32)
    return out
